# revision 1
# baseline (speedup 1.0000x reference)
"""MemN2N kernel for 8 Trainium2 NeuronCores.

Math note: in the reference, the attention weights p = mem_mask do not depend
on the query, so every hop adds the same x @ W.  The whole module collapses to

    lengths[b] = sum(masking[b])
    query0[b]  = sentences[b, lengths[b]-1]
    x[b]       = sum_{s < lengths[b]-1} sentences[b, s, :]
    out        = query0 + hops * (x @ W)          # [B, 1, D]

The memory-bound part is the masked row-sum x.  Sharding: batches are
bin-packed 8-per-core (balanced by valid-row count); the host packs only the
valid rows of each batch (padded with zero rows to a 256 multiple) into a flat
row stream per core plus a one-hot row->slot selector, so each core's
TensorEngine computes all 8 of its batch sums in a single PSUM accumulation
chain of float32r matmuls:

    x_ps[8, 512] += sel2[128, 8].T @ a_pair[128, 512]     (1 cyc/row, fp22)

where a_pair holds two 128-row chunks side by side (so each matmul streams 512
columns) and x_ps keeps two half-sums that are added at the end.  Data is
DMA'd in 1 MB tiles alternating between the two HWDGE engines (sync/scalar) to
saturate HBM, plus one 256-row-granular remainder tile so cores don't round up
to a full extra MB.  Tail (tiny): transpose x via the PE, two matmuls with
(hops*W), add the query rows, DMA out [8, D] per core.
"""

import math

import numpy as np

import concourse.bass as bass
import concourse.mybir as mybir
from concourse import bacc
from concourse.bass_utils import run_bass_kernel_spmd
from concourse.tile import TileContext

N_CORES = 8
SLOTS = 8  # batches per core
P = 128  # SBUF partitions / rows per chunk
D = 256  # model dim (hardcoded for this problem)
PAIR = 2 * P  # rows per matmul (two chunks side by side)
CPT = 8  # chunks per DMA tile
TILE_ROWS = CPT * P  # 1024 rows = 1 MB per DMA
PPT = CPT // 2  # pair-blocks (= matmuls) per DMA tile

_nc_cache: dict = {}


def _build_bass(T: int, rem: int):
    """Bass program for one core: T DMA tiles of 1024 packed rows plus an
    optional remainder tile of `rem` pair-blocks (256 rows each)."""
    f32 = mybir.dt.float32
    f32r = mybir.dt.float32r
    T2 = T * PPT + rem  # pair blocks

    nc = bacc.Bacc(None)
    a_d = nc.dram_tensor("a", [T, P, CPT * D], f32r, kind="ExternalInput")
    if rem:
        ar_d = nc.dram_tensor("ar", [P, rem * 2 * D], f32r, kind="ExternalInput")
    sel_d = nc.dram_tensor("sel", [P, T2 * SLOTS], f32r, kind="ExternalInput")
    q_d = nc.dram_tensor("q", [SLOTS, D], f32, kind="ExternalInput")
    w_d = nc.dram_tensor("w", [2, P, D], f32, kind="ExternalInput")
    id_d = nc.dram_tensor("id8", [SLOTS, SLOTS], f32, kind="ExternalInput")
    out_d = nc.dram_tensor("out", [SLOTS, D], f32, kind="ExternalOutput")

    with TileContext(nc) as tc:
        with (
            tc.tile_pool(name="const", bufs=1) as cpool,
            tc.tile_pool(name="a", bufs=10) as apool,
            tc.tile_pool(name="acc", bufs=1, space=bass.MemorySpace.PSUM) as accpool,
            tc.tile_pool(name="ps2", bufs=2, space=bass.MemorySpace.PSUM) as ps2pool,
            tc.tile_pool(name="tail", bufs=1) as tpool,
        ):
            # sel + tail constants on the scalar queue so the first a-tile
            # DMAs start immediately on the sync queue
            sel_sb = cpool.tile([P, T2 * SLOTS], f32r)
            nc.scalar.dma_start(out=sel_sb[:], in_=sel_d[:])
            w_sb = cpool.tile([P, 2 * D], f32)
            for h in range(2):
                nc.scalar.dma_start(out=w_sb[:, h * D : (h + 1) * D], in_=w_d[h])
            q_sb = cpool.tile([SLOTS, D], f32)
            nc.scalar.dma_start(out=q_sb[:], in_=q_d[:])
            id_sb = cpool.tile([SLOTS, SLOTS], f32)
            nc.scalar.dma_start(out=id_sb[:], in_=id_d[:])

            # Consume the sel DMA's semaphore with a throwaway PE matmul so
            # loop matmuls don't accumulate extra sync waits.
            warm_ps = ps2pool.tile([SLOTS, SLOTS], f32, tag="warm")
            nc.tensor.matmul(
                warm_ps[:],
                lhsT=sel_sb[:, 0:SLOTS],
                rhs=sel_sb[:, 0:SLOTS],
                start=True,
                stop=True,
            )

            # ---- masked row-sum: x_ps[slot, 0:256/256:512] = even/odd chunk
            # half-sums over all packed rows ----
            x_ps = accpool.tile([SLOTS, 2 * D], f32)
            for t in range(T):
                a_sb = apool.tile([P, CPT * D], f32r)
                eng = nc.sync if t % 2 == 0 else nc.scalar
                eng.dma_start(out=a_sb[:], in_=a_d[t])
                for g in range(PPT):
                    k2 = t * PPT + g
                    nc.tensor.matmul(
                        x_ps[:],
                        lhsT=sel_sb[:, k2 * SLOTS : (k2 + 1) * SLOTS],
                        rhs=a_sb[:, g * 2 * D : (g + 1) * 2 * D],
                        start=(k2 == 0),
                        stop=(k2 == T2 - 1),
                    )
            if rem:
                ar_sb = apool.tile([P, rem * 2 * D], f32r, tag="ar")
                (nc.sync if T % 2 == 0 else nc.scalar).dma_start(
                    out=ar_sb[:], in_=ar_d[:]
                )
                for g in range(rem):
                    k2 = T * PPT + g
                    nc.tensor.matmul(
                        x_ps[:],
                        lhsT=sel_sb[:, k2 * SLOTS : (k2 + 1) * SLOTS],
                        rhs=ar_sb[:, g * 2 * D : (g + 1) * 2 * D],
                        start=(k2 == 0),
                        stop=(k2 == T2 - 1),
                    )

            # ---- tail: out = q + x @ (hops*W) ----
            x_sb = tpool.tile([SLOTS, D], f32)
            nc.vector.tensor_copy(out=x_sb[:], in_=x_ps[:, 0:D])
            nc.vector.tensor_add(out=x_sb[:], in0=x_sb[:], in1=x_ps[:, D : 2 * D])
            xT_sb = tpool.tile([P, 2 * SLOTS], f32)
            for h in range(2):
                tp_ps = ps2pool.tile([P, SLOTS], f32)
                nc.tensor.transpose(tp_ps[:], x_sb[:, h * P : (h + 1) * P], id_sb[:])
                nc.vector.tensor_copy(
                    out=xT_sb[:, h * SLOTS : (h + 1) * SLOTS], in_=tp_ps[:]
                )
            out_ps = ps2pool.tile([SLOTS, D], f32)
            for h in range(2):
                nc.tensor.matmul(
                    out_ps[:],
                    lhsT=xT_sb[:, h * SLOTS : (h + 1) * SLOTS],
                    rhs=w_sb[:, h * D : (h + 1) * D],
                    start=(h == 0),
                    stop=(h == 1),
                )
            out_sb = tpool.tile([SLOTS, D], f32)
            nc.vector.tensor_add(out=out_sb[:], in0=q_sb[:], in1=out_ps[:])
            nc.sync.dma_start(out=out_d[:], in_=out_sb[:])

    nc.compile()  # bacc legalization: splits >1-wait instructions etc.
    return nc


def _prepare(sentences, masking, W, hops):
    """Host-side sharding: lengths, query gather, bin-packing, row packing."""
    sentences = np.ascontiguousarray(np.asarray(sentences), dtype=np.float32)
    masking = np.asarray(masking)
    W = np.ascontiguousarray(np.asarray(W), dtype=np.float32)
    hops = int(np.asarray(hops))

    B, S, Dd = sentences.shape
    assert Dd == D and B % N_CORES == 0
    lengths = masking.astype(np.int64).sum(axis=-1)  # [B]
    qidx = np.clip(lengths - 1, 0, S - 1)
    query = sentences[np.arange(B), qidx]  # [B, D]
    mem_len = np.clip(lengths - 1, 0, S).astype(np.int64)  # valid memory rows
    # pad each batch's row block to a PAIR multiple so every pair-block
    # belongs to exactly one batch (uniform selector; pad rows are zero data)
    padded = ((mem_len + PAIR - 1) // PAIR) * PAIR

    # Bin-pack batches: exactly SLOTS per core, balancing sum(padded) (LPT).
    order = np.argsort(-padded, kind="stable")
    core_load = [0] * N_CORES
    core_batches: list[list[int]] = [[] for _ in range(N_CORES)]
    for b in order:
        open_cores = [c for c in range(N_CORES) if len(core_batches[c]) < SLOTS]
        c = min(open_cores, key=lambda c: core_load[c])
        core_batches[c].append(int(b))
        core_load[c] += int(padded[b])

    # T full 1MB tiles plus a 256-row-granular remainder tile to avoid
    # rounding every core up to a full extra MB
    max_load = max(core_load)
    if max_load <= TILE_ROWS:
        T, rem = 1, 0
    else:
        T = max_load // TILE_ROWS
        rem = (max_load - T * TILE_ROWS + PAIR - 1) // PAIR
    R = T * TILE_ROWS + rem * PAIR
    T2 = R // PAIR

    # fold the hop count into W: out = q + hops * (x @ W) = q + x @ (hops*W)
    w_split = (W * np.float32(hops)).reshape(2, P, D)
    id8 = np.eye(SLOTS, dtype=np.float32)
    in_maps = []
    for c in range(N_CORES):
        A = np.zeros((R, D), dtype=np.float32)
        sel2 = np.zeros((T2, SLOTS), dtype=np.float32)
        pos = 0
        for j, b in enumerate(core_batches[c]):
            m = int(mem_len[b])
            pp = int(padded[b])
            if m > 0:
                A[pos : pos + m] = sentences[b, :m]
                sel2[pos // PAIR : (pos + pp) // PAIR, j] = 1.0
            pos += pp
        # device expects tile t, partition p, chunk cc: row t*TILE_ROWS+cc*P+p
        Afull = A[: T * TILE_ROWS]
        a_dev = np.ascontiguousarray(
            Afull.reshape(T, CPT, P, D).transpose(0, 2, 1, 3).reshape(T, P, CPT * D)
        )
        sel_dev = np.ascontiguousarray(
            np.broadcast_to(sel2.reshape(1, T2 * SLOTS), (P, T2 * SLOTS))
        )
        im = {
            "a": a_dev,
            "sel": sel_dev,
            "q": np.ascontiguousarray(query[core_batches[c]]),
            "w": w_split,
            "id8": id8,
        }
        if rem:
            Ar = A[T * TILE_ROWS :]
            im["ar"] = np.ascontiguousarray(
                Ar.reshape(rem * 2, P, D).transpose(1, 0, 2).reshape(P, rem * 2 * D)
            )
        in_maps.append(im)
    return in_maps, core_batches, (T, rem), hops, B


def _run(sentences, masking, W, hops, trace=False):
    in_maps, core_batches, key, hops_i, B = _prepare(sentences, masking, W, hops)
    if key not in _nc_cache:
        _nc_cache[key] = _build_bass(*key)
    nc = _nc_cache[key]
    res = run_bass_kernel_spmd(
        nc, in_maps, core_ids=list(range(N_CORES)), trace=trace
    )
    out = np.empty((B, 1, D), dtype=np.float32)
    for c in range(N_CORES):
        r = res.results[c]["out"]
        for j, b in enumerate(core_batches[c]):
            out[b, 0] = r[j]
    return out, res


def kernel(sentences, masking, W, hops):
    out, _ = _run(sentences, masking, W, hops)
    return out



# revision 4
# speedup vs baseline: 2.0127x; 2.0127x over previous
"""MemN2N kernel for 8 Trainium2 NeuronCores — fp8(e3m4) streaming version.

Math: the attention weights in the reference don't depend on the query, so the
module collapses to

    lengths[b] = sum(masking[b]);  q0[b] = sentences[b, lengths[b]-1]
    x[b]       = sum_{s < lengths[b]-1} sentences[b, s, :]
    out        = q0 + hops * (x @ W)                      # [B, 1, D]

The only heavy part is the masked row-sum x — pure HBM streaming.  Design:

* Rows are quantized host-side to float8_e3m4 (4 mantissa bits); the final
  rel-err this induces is ~1.3e-2 (measured against fp32), under the 2e-2
  tolerance, and it cuts DMA traffic 4x vs fp32.
* Lane packing: each batch's valid rows are split into lanes of G=8
  consecutive rows (last lane zero-padded).  The global lane stream is cut
  into 8 equal per-core spans at lane granularity, so core loads are balanced
  to within one lane and batches may split across cores (their partial sums
  are combined on the host; out = q + x@W is linear in x).
* A group = 128 lanes = one SBUF partition block.  Within a group, lane p's
  8 rows sit at partition p, depth slots 0..7 (chunks).  A per-group one-hot
  selector sel[p, j] (lane -> local batch slot) turns the PE into a segmented
  row-summer:  x_ps[16, 512] += sel[128,16].T @ chunk_pair[128, 512].
* PE streams 128 B/cycle of fp8, slightly slower than DMA (360 GB/s), so the
  DVE pre-adds depth pairs (0+1, 2+3) of every lane into fp16 "merged" chunks
  — half the group's rows then cost the PE half the cycles.  Per group the PE
  runs 2 raw fp8 matmuls (depths 4..7) + 1 merged fp16 matmul.
* 1 MB DMA tiles (4 groups), alternating sync/scalar HWDGE queues; constants
  ride the gpsimd (SWDGE) queue; a burst of dummy matmuls ramps the PE
  p-state while the first tile is in flight.
* Tail per core: fold hops into W (fp16), transpose x via the PE, two
  matmuls, add the gathered fp32 query rows, DMA [16, 256] out.
"""

import numpy as np
import ml_dtypes

import concourse.bass as bass
import concourse.mybir as mybir
from concourse import bacc
from concourse.bass_utils import run_bass_kernel_spmd
from concourse.tile import TileContext

N_CORES = 8
SLOTS = 16  # max distinct batches per core span
P = 128  # SBUF partitions = lanes per group
D = 256  # model dim
G = 8  # rows per lane (depth)
GPT = 4  # groups per full DMA tile (1 MB fp8)
LANES_PER_TILE = GPT * P
WARM = 10  # PE p-state warmup matmuls

F8 = mybir.dt.float8e3
F16 = mybir.dt.float16
F32 = mybir.dt.float32
NP8 = ml_dtypes.float8_e3m4

_nc_cache: dict = {}


def _build_bass(T: int, rems: tuple):
    """One-core program: T full 1MB tiles + len(rems) partial groups."""
    NGF = T * GPT  # full groups
    NG = NGF + len(rems)

    nc = bacc.Bacc(None)
    a_d = nc.dram_tensor("a", [T, P, 8 * D * GPT], F8, kind="ExternalInput")
    ar_d = [
        nc.dram_tensor(f"ar{k}", [r, G * D], F8, kind="ExternalInput")
        for k, r in enumerate(rems)
    ]
    sel8_d = nc.dram_tensor("sel8", [P, NG * SLOTS], F8, kind="ExternalInput")
    sel16_d = nc.dram_tensor("sel16", [P, NGF * SLOTS], F16, kind="ExternalInput")
    q_d = nc.dram_tensor("q", [SLOTS, D], F32, kind="ExternalInput")
    w_d = nc.dram_tensor("w", [2, P, D], F16, kind="ExternalInput")
    id_d = nc.dram_tensor("id16", [SLOTS, SLOTS], F32, kind="ExternalInput")
    out_d = nc.dram_tensor("out", [SLOTS, D], F32, kind="ExternalOutput")

    TILE_C = 8 * D * GPT  # 8192 cols: [in0 2048 | in1 2048 | raw 4096]
    RAW0 = 4 * D * GPT  # 4096: start of raw block

    with TileContext(nc) as tc:
        with (
            tc.tile_pool(name="const", bufs=1) as cpool,
            tc.tile_pool(name="a", bufs=3) as apool,
            tc.tile_pool(name="m", bufs=3) as mpool,
            tc.tile_pool(name="acc", bufs=1, space=bass.MemorySpace.PSUM) as accpool,
            tc.tile_pool(name="wps", bufs=1, space=bass.MemorySpace.PSUM) as wpspool,
            tc.tile_pool(name="ps2", bufs=2, space=bass.MemorySpace.PSUM) as ps2pool,
            tc.tile_pool(name="tail", bufs=1) as tpool,
        ):
            # constants on the gpsimd/SWDGE queue; payload owns sync+scalar
            sel8_sb = cpool.tile([P, NG * SLOTS], F8)
            nc.gpsimd.dma_start(out=sel8_sb[:], in_=sel8_d[:])
            sel16_sb = cpool.tile([P, NGF * SLOTS], F16)
            nc.gpsimd.dma_start(out=sel16_sb[:], in_=sel16_d[:])
            w_sb = cpool.tile([P, 2 * D], F16)
            for h in range(2):
                nc.gpsimd.dma_start(out=w_sb[:, h * D : (h + 1) * D], in_=w_d[h])
            q_sb = cpool.tile([SLOTS, D], F32)
            nc.gpsimd.dma_start(out=q_sb[:], in_=q_d[:])
            id_sb = cpool.tile([SLOTS, SLOTS], F32)
            nc.gpsimd.dma_start(out=id_sb[:], in_=id_d[:])

            # PE p-state warmup on a zeroed tile (own psum bank, never read)
            warm_sb = cpool.tile([P, 512], F16)
            nc.vector.memset(warm_sb[:], 0.0)
            warm_ps = wpspool.tile([SLOTS, 512], F32)
            for _ in range(WARM):
                nc.tensor.matmul(
                    warm_ps[:],
                    lhsT=warm_sb[:, 0:SLOTS],
                    rhs=warm_sb[:],
                    start=True,
                    stop=True,
                )

            # ---- streamed masked row-sum ----
            x_ps = accpool.tile([SLOTS, 2 * D], F32)
            n_mm = T * GPT * 3 + len(rems) * (G // 2)  # total accumulation chain
            k_mm = 0
            for t in range(T):
                a_sb = apool.tile([P, TILE_C], F8)
                eng = nc.sync if t % 2 == 0 else nc.scalar
                eng.dma_start(out=a_sb[:], in_=a_d[t])
                m_sb = mpool.tile([P, 2 * D * GPT], F16)
                nc.vector.tensor_add(
                    out=m_sb[:],
                    in0=a_sb[:, 0 : 2 * D * GPT],
                    in1=a_sb[:, 2 * D * GPT : 4 * D * GPT],
                )
                # raw fp8 matmuls (depths 4..7), ready as soon as the DMA lands
                for g in range(GPT):
                    gg = t * GPT + g
                    for j in range(2):
                        nc.tensor.matmul(
                            x_ps[:],
                            lhsT=sel8_sb[:, gg * SLOTS : (gg + 1) * SLOTS],
                            rhs=a_sb[:, RAW0 + g * 4 * D + j * 2 * D :
                                     RAW0 + g * 4 * D + (j + 1) * 2 * D],
                            start=(k_mm == 0),
                            stop=(k_mm == n_mm - 1),
                        )
                        k_mm += 1
                # merged fp16 matmuls (depths 0..3 pre-added by the DVE)
                for g in range(GPT):
                    gg = t * GPT + g
                    nc.tensor.matmul(
                        x_ps[:],
                        lhsT=sel16_sb[:, gg * SLOTS : (gg + 1) * SLOTS],
                        rhs=m_sb[:, g * 2 * D : (g + 1) * 2 * D],
                        start=(k_mm == 0),
                        stop=(k_mm == n_mm - 1),
                    )
                    k_mm += 1
            for k, r in enumerate(rems):
                ar_sb = apool.tile([r, G * D], F8, tag=f"ar{k}")
                (nc.sync if (T + k) % 2 == 0 else nc.scalar).dma_start(
                    out=ar_sb[:], in_=ar_d[k][:]
                )
                gg = NGF + k
                for j in range(G // 2):
                    nc.tensor.matmul(
                        x_ps[:],
                        lhsT=sel8_sb[0:r, gg * SLOTS : (gg + 1) * SLOTS],
                        rhs=ar_sb[:, j * 2 * D : (j + 1) * 2 * D],
                        start=(k_mm == 0),
                        stop=(k_mm == n_mm - 1),
                    )
                    k_mm += 1
            assert k_mm == n_mm

            # ---- tail: out = q + x @ (hops*W) ----
            xh_sb = tpool.tile([SLOTS, D], F32)
            nc.vector.tensor_copy(out=xh_sb[:], in_=x_ps[:, 0:D])
            x2_sb = tpool.tile([SLOTS, D], F32)
            nc.vector.tensor_add(out=x2_sb[:], in0=xh_sb[:], in1=x_ps[:, D : 2 * D])
            xT_sb = tpool.tile([P, 2 * SLOTS], F16)
            for h in range(2):
                tp_ps = ps2pool.tile([P, SLOTS], F32)
                nc.tensor.transpose(
                    tp_ps[:], x2_sb[:, h * P : (h + 1) * P], id_sb[:]
                )
                nc.vector.tensor_copy(
                    out=xT_sb[:, h * SLOTS : (h + 1) * SLOTS], in_=tp_ps[:]
                )
            out_ps = ps2pool.tile([SLOTS, D], F32)
            for h in range(2):
                nc.tensor.matmul(
                    out_ps[:],
                    lhsT=xT_sb[:, h * SLOTS : (h + 1) * SLOTS],
                    rhs=w_sb[:, h * D : (h + 1) * D],
                    start=(h == 0),
                    stop=(h == 1),
                )
            out_sb = tpool.tile([SLOTS, D], F32)
            nc.vector.tensor_add(out=out_sb[:], in0=q_sb[:], in1=out_ps[:])
            nc.sync.dma_start(out=out_d[:], in_=out_sb[:])

    nc.compile()
    return nc


def _prepare(sentences, masking, W, hops):
    """Host sharding: quantize valid rows to fp8, lane-pack, split into 8
    balanced contiguous spans, build per-core tile/selector arrays."""
    sentences = np.asarray(sentences)
    masking = np.asarray(masking)
    W = np.ascontiguousarray(np.asarray(W), dtype=np.float32)
    hops = int(np.asarray(hops))

    B, S, Dd = sentences.shape
    assert Dd == D
    lengths = masking.astype(np.int64).sum(axis=-1)  # [B]
    qidx = np.clip(lengths - 1, 0, S - 1)
    query = np.ascontiguousarray(
        sentences[np.arange(B), qidx], dtype=np.float32
    )  # [B, D]
    mem = np.clip(lengths - 1, 0, S).astype(np.int64)  # valid memory rows

    lanes_b = -(-mem // G)  # ceil
    lane_ofs = np.concatenate([[0], np.cumsum(lanes_b)])
    L = int(lane_ofs[-1])
    base = -(-L // N_CORES)
    T = base // LANES_PER_TILE
    rem = base - T * LANES_PER_TILE
    rems = []
    while rem > 0:
        rems.append(min(rem, P))
        rem -= min(rem, P)
    rems = tuple(rems)
    NGF = T * GPT
    NG = NGF + len(rems)

    # quantize + lane-pack all batches into one [base*8, G, D] fp8 array
    Apad = np.zeros((base * N_CORES, G, D), dtype=NP8)
    rows_flat = Apad.reshape(-1, D)
    for b in range(B):
        m = int(mem[b])
        if m:
            rows_flat[lane_ofs[b] * G : lane_ofs[b] * G + m] = sentences[
                b, :m
            ].astype(NP8)

    # per-lane batch id; SLOTS bookkeeping per core
    lane_batch = np.repeat(np.arange(B), lanes_b)
    lane_batch = np.concatenate(
        [lane_batch, np.full(base * N_CORES - L, -1, dtype=np.int64)]
    )
    w_dev = np.ascontiguousarray(
        (W * np.float32(hops)).astype(np.float16).reshape(2, P, D)
    )
    id_dev = np.eye(SLOTS, dtype=np.float32)

    in_maps = []
    core_slots: list[list[int]] = []  # per core: slot j -> batch id
    for c in range(N_CORES):
        span = lane_batch[c * base : (c + 1) * base]
        bids: list[int] = []
        slot_of = {}
        for b in span:
            if b >= 0 and b not in slot_of:
                slot_of[b] = len(bids)
                bids.append(int(b))
        assert len(bids) <= SLOTS, f"core {c} needs {len(bids)} slots"
        core_slots.append(bids)

        sel8 = np.zeros((P, NG * SLOTS), dtype=NP8)
        sel16 = np.zeros((P, NGF * SLOTS), dtype=np.float16)
        for g in range(NG):
            g0 = g * P
            gl = P if g < NGF else rems[g - NGF]
            for p in range(gl):
                li = g0 + p
                if li < base and span[li] >= 0:
                    j = slot_of[int(span[li])]
                    sel8[p, g * SLOTS + j] = 1.0
                    if g < NGF:
                        sel16[p, g * SLOTS + j] = 1.0

        lanes = Apad[c * base : (c + 1) * base]  # [base, G, D]
        lt = lanes[: T * LANES_PER_TILE].reshape(T, GPT, P, G, D)
        in0 = lt[:, :, :, 0:4:2, :].transpose(0, 2, 1, 3, 4).reshape(T, P, -1)
        in1 = lt[:, :, :, 1:4:2, :].transpose(0, 2, 1, 3, 4).reshape(T, P, -1)
        raw = lt[:, :, :, 4:8, :].transpose(0, 2, 1, 3, 4).reshape(T, P, -1)
        a_dev = np.ascontiguousarray(np.concatenate([in0, in1, raw], axis=2))

        qmat = np.zeros((SLOTS, D), dtype=np.float32)
        for j, b in enumerate(bids):
            # q rides with the core owning the batch's first lane
            if int(lane_ofs[b]) // base == c or (
                lanes_b[b] == 0 and c == 0
            ):
                qmat[j] = query[b]

        im = {
            "a": a_dev,
            "sel8": sel8,
            "sel16": sel16,
            "q": qmat,
            "w": w_dev,
            "id16": id_dev,
        }
        pos = T * LANES_PER_TILE
        for k, r in enumerate(rems):
            im[f"ar{k}"] = np.ascontiguousarray(
                lanes[pos : pos + r].reshape(r, G * D)
            )
            pos += r
        in_maps.append(im)
    return in_maps, core_slots, (T, rems), query, mem, hops, B


def _run(sentences, masking, W, hops, trace=False):
    in_maps, core_slots, key, query, mem, hops_i, B = _prepare(
        sentences, masking, W, hops
    )
    if key not in _nc_cache:
        _nc_cache[key] = _build_bass(*key)
    nc = _nc_cache[key]
    res = run_bass_kernel_spmd(
        nc, in_maps, core_ids=list(range(N_CORES)), trace=trace
    )
    out = np.zeros((B, 1, D), dtype=np.float32)
    for c in range(N_CORES):
        r = res.results[c]["out"]
        for j, b in enumerate(core_slots[c]):
            out[b, 0] += r[j]
    for b in range(B):
        if mem[b] == 0:  # no memory rows: out = query (never had a slot)
            out[b, 0] = query[b]
    return out, res


def kernel(sentences, masking, W, hops):
    out, _ = _run(sentences, masking, W, hops)
    return out


# revision 12
# speedup vs baseline: 2.2070x; 1.0965x over previous
"""MemN2N kernel for 8 Trainium2 NeuronCores — fp8(e3m4) streaming version.

Math: the attention weights in the reference don't depend on the query, so the
module collapses to

    lengths[b] = sum(masking[b]);  q0[b] = sentences[b, lengths[b]-1]
    x[b]       = sum_{s < lengths[b]-1} sentences[b, s, :]
    out        = q0 + hops * (x @ W)                      # [B, 1, D]

The only heavy part is the masked row-sum x — pure HBM streaming.  Design:

* Rows are quantized host-side to float8_e3m4 (4 mantissa bits); the final
  rel-err this induces is ~1.3e-2 (measured against fp32), under the 2e-2
  tolerance, and it cuts DMA traffic 4x vs fp32.
* Lane packing: each batch's valid rows are split into lanes of G=8
  consecutive rows (last lane zero-padded).  The global lane stream is cut
  into 8 equal per-core spans at lane granularity, so core loads are balanced
  to within one lane and batches may split across cores (their partial sums
  are combined on the host; out = q + x@W is linear in x).
* A group = 128 lanes = one SBUF partition block.  Within a group, lane p's
  8 rows sit at partition p, depth slots 0..7 (chunks).  A per-group one-hot
  selector sel[p, j] (lane -> local batch slot) turns the PE into a segmented
  row-summer:  x_ps[16, 512] += sel[128,16].T @ chunk_pair[128, 512].
* PE streams 128 B/cycle of fp8, slightly slower than DMA (360 GB/s), so the
  DVE pre-adds depth pairs (0+1, 2+3) of every lane into fp16 "merged" chunks
  — half those rows then cost the PE half the cycles.  The LAST tile (and the
  remainder) stay all-raw so nothing on the end-of-stream critical path waits
  for the DVE; merged matmuls for tile t are issued after tile t+1's raw ones
  to give the DVE a full tile of slack.
* 512 KB DMA tiles (2 groups), alternating sync/scalar HWDGE queues;
  constants + remainder ride the gpsimd (SWDGE) queue; a burst of dummy
  matmuls on an uninitialized tile ramps the PE p-state during NEFF startup.
* Tail per core: fold hops into W (fp16), transpose x via the PE, two
  matmuls, add the gathered fp32 query rows, DMA [16, 256] out.
"""

import numpy as np
import ml_dtypes

import concourse.bass as bass
import concourse.mybir as mybir
from concourse import bacc
from concourse.bass_utils import run_bass_kernel_spmd
from concourse.tile import TileContext

N_CORES = 8
SLOTS = 16  # max distinct batches per core span
P = 128  # SBUF partitions = lanes per group
D = 256  # model dim
G = 8  # rows per lane (depth)
GPT = 2  # groups per full DMA tile (512 KB fp8)
LANES_PER_TILE = GPT * P
WARM = 10  # PE p-state warmup matmuls

F8 = mybir.dt.float8e3
F16 = mybir.dt.float16
F32 = mybir.dt.float32
NP8 = ml_dtypes.float8_e3m4

_nc_cache: dict = {}


def _build_bass(T: int, rems: tuple):
    """One-core program: T 512KB tiles + len(rems) partial remainder groups.
    Tiles 0..T-2 run the DVE pre-add on depths 0..3; the last tile is all-raw."""
    NGF = T * GPT  # full groups
    NG = NGF + len(rems)
    n_merge_tiles = max(T - 1, 0)

    nc = bacc.Bacc(None)
    TILE_C = 8 * D * GPT  # 4096 cols: [in0 1024 | in1 1024 | raw 2048]
    RAW0 = 4 * D * GPT
    a_d = nc.dram_tensor("a", [T, P, TILE_C], F8, kind="ExternalInput")
    ar_d = [
        nc.dram_tensor(f"ar{k}", [r, G * D], F8, kind="ExternalInput")
        for k, r in enumerate(rems)
    ]
    sel8_d = nc.dram_tensor("sel8", [P, NG * SLOTS], F8, kind="ExternalInput")
    sel16_d = nc.dram_tensor("sel16", [P, NGF * SLOTS], F16, kind="ExternalInput")
    w16_d = nc.dram_tensor("w16", [P, 2 * D], F16, kind="ExternalInput")
    c32_d = nc.dram_tensor("c32", [SLOTS, D + SLOTS], F32, kind="ExternalInput")
    out_d = nc.dram_tensor("out", [SLOTS, D], F32, kind="ExternalOutput")

    with TileContext(nc) as tc:
        with (
            tc.tile_pool(name="const", bufs=1) as cpool,
            tc.tile_pool(name="a", bufs=1) as apool,
            tc.tile_pool(name="m", bufs=1) as mpool,
            tc.tile_pool(name="acc", bufs=1, space=bass.MemorySpace.PSUM) as accpool,
            tc.tile_pool(name="wps", bufs=1, space=bass.MemorySpace.PSUM) as wpspool,
            tc.tile_pool(name="ps2", bufs=2, space=bass.MemorySpace.PSUM) as ps2pool,
            tc.tile_pool(name="tail", bufs=1) as tpool,
        ):
            # selectors + remainder go FIRST on the sync queue (tiny
            # transfers, ahead of the payload flood on the shared DMA
            # engines); payload tiles start immediately on scalar.  Tail-only
            # constants + the warm memset ride the gpsimd/SWDGE queue.
            sel8_sb = cpool.tile([P, NG * SLOTS], F8)
            nc.sync.dma_start(out=sel8_sb[:], in_=sel8_d[:])
            sel16_sb = cpool.tile([P, NGF * SLOTS], F16)
            nc.sync.dma_start(out=sel16_sb[:], in_=sel16_d[:])
            ar_sb = []
            for k, r in enumerate(rems):
                t_ = cpool.tile([r, G * D], F8, tag=f"ar{k}")
                nc.sync.dma_start(out=t_[:], in_=ar_d[k][:])
                ar_sb.append(t_)
            warm_sb = cpool.tile([P, 512], F16)
            nc.gpsimd.memset(warm_sb[:], 0.0)
            w_sb = cpool.tile([P, 2 * D], F16)
            nc.gpsimd.dma_start(out=w_sb[:], in_=w16_d[:])
            c32_sb = cpool.tile([SLOTS, D + SLOTS], F32)
            nc.gpsimd.dma_start(out=c32_sb[:], in_=c32_d[:])
            q_sb = c32_sb[:, 0:D]
            id_sb = c32_sb[:, D : D + SLOTS]

            # PE p-state warmup: throwaway psum bank, never read
            warm_ps = wpspool.tile([SLOTS, 512], F32)
            for _ in range(WARM):
                nc.tensor.matmul(
                    warm_ps[:],
                    lhsT=warm_sb[:, 0:SLOTS],
                    rhs=warm_sb[:],
                    start=True,
                    stop=True,
                )

            # ---- streamed masked row-sum (single psum accumulation chain).
            # Jobs are collected in PE issue order so start/stop land on the
            # first/last matmul of the chain.
            x_ps = accpool.tile([SLOTS, 2 * D], F32)
            jobs = []  # (lhsT, rhs) in PE issue order

            a_sb = [None] * T
            m_sb = [None] * n_merge_tiles
            pend_merge = []  # deferred merged-matmul jobs, one tile behind
            for t in range(T):
                a_sb[t] = apool.tile([P, TILE_C], F8)
                nc.scalar.dma_start(out=a_sb[t][:], in_=a_d[t])
                merged = t < n_merge_tiles
                if merged:
                    m_sb[t] = mpool.tile([P, 2 * D * GPT], F16)
                    nc.vector.tensor_add(
                        out=m_sb[t][:],
                        in0=a_sb[t][:, 0 : 2 * D * GPT],
                        in1=a_sb[t][:, 2 * D * GPT : RAW0],
                    )
                # raw matmuls of this tile (depths 4..7 from the raw block)
                for g in range(GPT):
                    gg = t * GPT + g
                    sel = sel8_sb[:, gg * SLOTS : (gg + 1) * SLOTS]
                    lo = RAW0 + g * 4 * D
                    if merged:
                        for j in range(2):
                            jobs.append(
                                (sel, a_sb[t][:, lo + j * 2 * D : lo + (j + 1) * 2 * D])
                            )
                # all-raw tile: depths 0..3 also stream straight from the
                # in0/in1 blocks (host emits the same [in0|in1|raw] layout)
                if not merged:
                    for g in range(GPT):
                        gg = t * GPT + g
                        sel = sel8_sb[:, gg * SLOTS : (gg + 1) * SLOTS]
                        jobs.append((sel, a_sb[t][:, g * 2 * D : (g + 1) * 2 * D]))
                        jobs.append(
                            (
                                sel,
                                a_sb[t][
                                    :, 2 * D * GPT + g * 2 * D : 2 * D * GPT + (g + 1) * 2 * D
                                ],
                            )
                        )
                        jobs.append(
                            (sel, a_sb[t][:, RAW0 + g * 4 * D : RAW0 + g * 4 * D + 2 * D])
                        )
                        jobs.append(
                            (
                                sel,
                                a_sb[t][
                                    :, RAW0 + g * 4 * D + 2 * D : RAW0 + (g + 1) * 4 * D
                                ],
                            )
                        )
                # deferred merged matmuls from the previous tile
                jobs.extend(pend_merge)
                pend_merge = []
                if merged:
                    for g in range(GPT):
                        gg = t * GPT + g
                        pend_merge.append(
                            (
                                sel16_sb[:, gg * SLOTS : (gg + 1) * SLOTS],
                                m_sb[t][:, g * 2 * D : (g + 1) * 2 * D],
                            )
                        )
            jobs.extend(pend_merge)
            # remainder groups last: tiny, data long since arrived
            for k, r in enumerate(rems):
                gg = NGF + k
                for j in range(G // 2):
                    jobs.append(
                        (
                            sel8_sb[0:r, gg * SLOTS : (gg + 1) * SLOTS],
                            ar_sb[k][:, j * 2 * D : (j + 1) * 2 * D],
                        )
                    )

            for i, (lhsT, rhs) in enumerate(jobs):
                nc.tensor.matmul(
                    x_ps[:],
                    lhsT=lhsT,
                    rhs=rhs,
                    start=(i == 0),
                    stop=(i == len(jobs) - 1),
                )

            # ---- tail: out = q + x @ (hops*W) ----
            xh_sb = tpool.tile([SLOTS, D], F32)
            nc.vector.tensor_copy(out=xh_sb[:], in_=x_ps[:, 0:D])
            x2_sb = tpool.tile([SLOTS, D], F32)
            nc.vector.tensor_add(out=x2_sb[:], in0=xh_sb[:], in1=x_ps[:, D : 2 * D])
            xT_sb = tpool.tile([P, 2 * SLOTS], F16)
            for h in range(2):
                tp_ps = ps2pool.tile([P, SLOTS], F32)
                nc.tensor.transpose(
                    tp_ps[:], x2_sb[:, h * P : (h + 1) * P], id_sb
                )
                nc.vector.tensor_copy(
                    out=xT_sb[:, h * SLOTS : (h + 1) * SLOTS], in_=tp_ps[:]
                )
            out_ps = ps2pool.tile([SLOTS, D], F32)
            for h in range(2):
                nc.tensor.matmul(
                    out_ps[:],
                    lhsT=xT_sb[:, h * SLOTS : (h + 1) * SLOTS],
                    rhs=w_sb[:, h * D : (h + 1) * D],
                    start=(h == 0),
                    stop=(h == 1),
                )
            out_sb = tpool.tile([SLOTS, D], F32)
            nc.vector.tensor_add(out=out_sb[:], in0=q_sb, in1=out_ps[:])
            nc.sync.dma_start(out=out_d[:], in_=out_sb[:])

    nc.compile()
    return nc


def _prepare(sentences, masking, W, hops):
    """Host sharding: quantize valid rows to fp8, lane-pack, split into 8
    balanced contiguous spans, build per-core tile/selector arrays."""
    sentences = np.asarray(sentences)
    masking = np.asarray(masking)
    W = np.ascontiguousarray(np.asarray(W), dtype=np.float32)
    hops = int(np.asarray(hops))

    B, S, Dd = sentences.shape
    assert Dd == D
    lengths = masking.astype(np.int64).sum(axis=-1)  # [B]
    qidx = np.clip(lengths - 1, 0, S - 1)
    query = np.ascontiguousarray(
        sentences[np.arange(B), qidx], dtype=np.float32
    )  # [B, D]
    mem = np.clip(lengths - 1, 0, S).astype(np.int64)  # valid memory rows

    lanes_b = -(-mem // G)  # ceil
    lane_ofs = np.concatenate([[0], np.cumsum(lanes_b)])
    L = int(lane_ofs[-1])
    base = -(-L // N_CORES)
    T = base // LANES_PER_TILE
    rem = base - T * LANES_PER_TILE
    rems = []
    while rem > 0:
        rems.append(min(rem, P))
        rem -= min(rem, P)
    rems = tuple(rems)
    NGF = T * GPT
    NG = NGF + len(rems)

    # quantize + lane-pack all batches into one [base*8, G, D] fp8 array
    Apad = np.zeros((base * N_CORES, G, D), dtype=NP8)
    rows_flat = Apad.reshape(-1, D)
    for b in range(B):
        m = int(mem[b])
        if m:
            rows_flat[lane_ofs[b] * G : lane_ofs[b] * G + m] = sentences[
                b, :m
            ].astype(NP8)

    # per-lane batch id
    lane_batch = np.repeat(np.arange(B), lanes_b)
    lane_batch = np.concatenate(
        [lane_batch, np.full(base * N_CORES - L, -1, dtype=np.int64)]
    )
    w16 = (W * np.float32(hops)).astype(np.float16).reshape(2, P, D)
    id_dev = np.eye(SLOTS, dtype=np.float32)

    in_maps = []
    core_slots: list[list[int]] = []
    for c in range(N_CORES):
        span = lane_batch[c * base : (c + 1) * base]
        bids: list[int] = []
        slot_of = {}
        for b in span:
            if b >= 0 and b not in slot_of:
                slot_of[b] = len(bids)
                bids.append(int(b))
        assert len(bids) <= SLOTS, f"core {c} needs {len(bids)} slots"
        core_slots.append(bids)

        sel8 = np.zeros((P, NG * SLOTS), dtype=NP8)
        sel16 = np.zeros((P, NGF * SLOTS), dtype=np.float16)
        for g in range(NG):
            g0 = g * P
            gl = P if g < NGF else rems[g - NGF]
            for p in range(gl):
                li = g0 + p
                if li < base and span[li] >= 0:
                    j = slot_of[int(span[li])]
                    sel8[p, g * SLOTS + j] = 1.0
                    if g < NGF:
                        sel16[p, g * SLOTS + j] = 1.0

        lanes = Apad[c * base : (c + 1) * base]  # [base, G, D]
        lt = lanes[: T * LANES_PER_TILE].reshape(T, GPT, P, G, D)
        in0 = lt[:, :, :, 0:4:2, :].transpose(0, 2, 1, 3, 4).reshape(T, P, -1)
        in1 = lt[:, :, :, 1:4:2, :].transpose(0, 2, 1, 3, 4).reshape(T, P, -1)
        raw = lt[:, :, :, 4:8, :].transpose(0, 2, 1, 3, 4).reshape(T, P, -1)
        a_dev = np.ascontiguousarray(np.concatenate([in0, in1, raw], axis=2))

        qmat = np.zeros((SLOTS, D), dtype=np.float32)
        for j, b in enumerate(bids):
            if int(lane_ofs[b]) // base == c:  # q rides with the owner core
                qmat[j] = query[b]
        c32 = np.concatenate([qmat, id_dev], axis=1)

        im = {
            "a": a_dev,
            "sel8": sel8,
            "sel16": sel16,
            "w16": np.ascontiguousarray(w16.transpose(1, 0, 2).reshape(P, 2 * D)),
            "c32": np.ascontiguousarray(c32),
        }
        pos = T * LANES_PER_TILE
        for k, r in enumerate(rems):
            im[f"ar{k}"] = np.ascontiguousarray(
                lanes[pos : pos + r].reshape(r, G * D)
            )
            pos += r
        in_maps.append(im)
    return in_maps, core_slots, (T, rems), query, mem, hops, B


def _run(sentences, masking, W, hops, trace=False):
    in_maps, core_slots, key, query, mem, hops_i, B = _prepare(
        sentences, masking, W, hops
    )
    if key not in _nc_cache:
        _nc_cache[key] = _build_bass(*key)
    nc = _nc_cache[key]
    res = run_bass_kernel_spmd(
        nc, in_maps, core_ids=list(range(N_CORES)), trace=trace
    )
    out = np.zeros((B, 1, D), dtype=np.float32)
    for c in range(N_CORES):
        r = res.results[c]["out"]
        for j, b in enumerate(core_slots[c]):
            out[b, 0] += r[j]
    for b in range(B):
        if mem[b] == 0:  # no memory rows: out = query (never had a slot)
            out[b, 0] = query[b]
    return out, res


def kernel(sentences, masking, W, hops):
    out, _ = _run(sentences, masking, W, hops)
    return out


# revision 16
# speedup vs baseline: 2.2857x; 1.0357x over previous
"""MemN2N kernel for 8 Trainium2 NeuronCores — fp8(e3m4) streaming version.

Math: the attention weights in the reference don't depend on the query, so the
module collapses to

    lengths[b] = sum(masking[b]);  q0[b] = sentences[b, lengths[b]-1]
    x[b]       = sum_{s < lengths[b]-1} sentences[b, s, :]
    out        = q0 + hops * (x @ W)                      # [B, 1, D]

The only heavy part is the masked row-sum x — pure HBM streaming.  Design:

* Rows are quantized host-side to float8_e3m4 (4 mantissa bits); the final
  rel-err this induces is ~1.3e-2 (measured against fp32), under the 2e-2
  tolerance, and it cuts DMA traffic 4x vs fp32.
* Lane packing: each batch's valid rows are split into lanes of G=8
  consecutive rows (last lane zero-padded).  The global lane stream is cut
  into 8 equal per-core spans at lane granularity, so core loads are balanced
  to within one lane and batches may split across cores (their partial sums
  are combined on the host; out = q + x@W is linear in x).
* A group = 128 lanes = one SBUF partition block.  Within a group, lane p's
  8 rows sit at partition p, depth slots 0..7 (chunks).  A per-group one-hot
  selector sel[p, j] (lane -> local batch slot) turns the PE into a segmented
  row-summer:  x_ps[16, 512] += sel[128,16].T @ chunk_pair[128, 512].
* PE streams 128 B/cycle of fp8, slightly slower than DMA (360 GB/s), so the
  DVE pre-adds depth pairs (0+1, 2+3) of every lane into fp16 "merged" chunks
  — half those rows then cost the PE half the cycles.  The LAST tile (and the
  remainder) stay all-raw so nothing on the end-of-stream critical path waits
  for the DVE; merged matmuls for tile t are issued after tile t+1's raw ones
  to give the DVE a full tile of slack.
* 512 KB DMA tiles (2 groups), alternating sync/scalar HWDGE queues;
  constants + remainder ride the gpsimd (SWDGE) queue; a burst of dummy
  matmuls on an uninitialized tile ramps the PE p-state during NEFF startup.
* Tail per core: fold hops into W (fp16), transpose x via the PE, two
  matmuls, add the gathered fp32 query rows, DMA [16, 256] out.
"""

import numpy as np
import ml_dtypes

import concourse.bass as bass
import concourse.mybir as mybir
from concourse import bacc
from concourse.bass_utils import run_bass_kernel_spmd
from concourse.tile import TileContext

N_CORES = 8
SLOTS = 16  # max distinct batches per core span
P = 128  # SBUF partitions = lanes per group
D = 256  # model dim
G = 8  # rows per lane (depth)
GPT = 2  # groups per full DMA tile (512 KB fp8)
LANES_PER_TILE = GPT * P
WARM = 10  # PE p-state warmup matmuls

F8 = mybir.dt.float8e3
F16 = mybir.dt.float16
F32 = mybir.dt.float32
NP8 = ml_dtypes.float8_e3m4

_nc_cache: dict = {}


def _build_bass(plan: tuple, rems: tuple):
    """One-core program: payload tiles of plan[t] groups each (the last two
    tiles are single-group so the end-of-stream PE burst is short) plus
    len(rems) partial remainder groups.  All tiles except the last two run
    the DVE pre-add on depths 0..3; the final tiles are all-raw."""
    T = len(plan)
    NGF = sum(plan)  # full groups
    NG = NGF + len(rems)
    n_merge_tiles = max(T - 2, 0)

    nc = bacc.Bacc(None)
    SEL_W = NG * SLOTS  # fp8 selector rides inside tile 0's dma
    coff = [0]
    for t_, g_ in enumerate(plan):
        coff.append(coff[-1] + (SEL_W if t_ == 0 else 0) + 8 * D * g_)
    a_d = nc.dram_tensor("a", [P, coff[-1]], F8, kind="ExternalInput")
    ar_d = [
        nc.dram_tensor(f"ar{k}", [r, G * D], F8, kind="ExternalInput")
        for k, r in enumerate(rems)
    ]
    sel16_d = nc.dram_tensor("sel16", [P, NGF * SLOTS], F16, kind="ExternalInput")
    w16_d = nc.dram_tensor("w16", [P, 2 * D], F16, kind="ExternalInput")
    c32_d = nc.dram_tensor("c32", [SLOTS, D + SLOTS], F32, kind="ExternalInput")
    out_d = nc.dram_tensor("out", [SLOTS, D], F32, kind="ExternalOutput")

    with TileContext(nc) as tc:
        with (
            tc.tile_pool(name="const", bufs=1) as cpool,
            tc.tile_pool(name="a", bufs=1) as apool,
            tc.tile_pool(name="m", bufs=1) as mpool,
            tc.tile_pool(name="acc", bufs=1, space=bass.MemorySpace.PSUM) as accpool,
            tc.tile_pool(name="wps", bufs=1, space=bass.MemorySpace.PSUM) as wpspool,
            tc.tile_pool(name="ps2", bufs=2, space=bass.MemorySpace.PSUM) as ps2pool,
            tc.tile_pool(name="tail", bufs=1) as tpool,
        ):
            # selectors + remainder go FIRST on the sync queue (tiny
            # transfers, ahead of the payload flood on the shared DMA
            # engines); payload tiles start immediately on scalar.  Tail-only
            # constants + the warm memset ride the gpsimd/SWDGE queue.
            sel16_sb = cpool.tile([P, NGF * SLOTS], F16)
            nc.sync.dma_start(out=sel16_sb[:], in_=sel16_d[:])
            ar_sb = []
            for k, r in enumerate(rems):
                t_ = cpool.tile([r, G * D], F8, tag=f"ar{k}")
                nc.sync.dma_start(out=t_[:], in_=ar_d[k][:])
                ar_sb.append(t_)
            warm_sb = cpool.tile([P, 512], F16)
            nc.gpsimd.memset(warm_sb[:], 0.0)
            w_sb = cpool.tile([P, 2 * D], F16)
            nc.gpsimd.dma_start(out=w_sb[:], in_=w16_d[:])
            c32_sb = cpool.tile([SLOTS, D + SLOTS], F32)
            nc.gpsimd.dma_start(out=c32_sb[:], in_=c32_d[:])
            q_sb = c32_sb[:, 0:D]
            id_sb = c32_sb[:, D : D + SLOTS]

            # PE p-state warmup: throwaway psum bank, never read
            warm_ps = wpspool.tile([SLOTS, 512], F32)
            for _ in range(WARM):
                nc.tensor.matmul(
                    warm_ps[:],
                    lhsT=warm_sb[:, 0:SLOTS],
                    rhs=warm_sb[:],
                    start=True,
                    stop=True,
                )

            # ---- streamed masked row-sum (single psum accumulation chain).
            # Jobs are collected in PE issue order so start/stop land on the
            # first/last matmul of the chain.
            x_ps = accpool.tile([SLOTS, 2 * D], F32)
            jobs = []  # (lhsT, rhs) in PE issue order

            a_sb = [None] * T
            m_sb = [None] * n_merge_tiles
            pend_merge = []  # deferred merged-matmul jobs, one tile behind
            for t in range(T):
                a_sb[t] = apool.tile([P, TILE_C], F8)
                nc.scalar.dma_start(out=a_sb[t][:], in_=a_d[t])
                merged = t < n_merge_tiles
                if merged:
                    m_sb[t] = mpool.tile([P, 2 * D * GPT], F16)
                    nc.vector.tensor_add(
                        out=m_sb[t][:],
                        in0=a_sb[t][:, 0 : 2 * D * GPT],
                        in1=a_sb[t][:, 2 * D * GPT : RAW0],
                    )
                # raw matmuls of this tile (depths 4..7 from the raw block)
                for g in range(GPT):
                    gg = t * GPT + g
                    sel = sel8_sb[:, gg * SLOTS : (gg + 1) * SLOTS]
                    lo = RAW0 + g * 4 * D
                    if merged:
                        for j in range(2):
                            jobs.append(
                                (sel, a_sb[t][:, lo + j * 2 * D : lo + (j + 1) * 2 * D])
                            )
                # all-raw tile: depths 0..3 also stream straight from the
                # in0/in1 blocks (host emits the same [in0|in1|raw] layout)
                if not merged:
                    for g in range(GPT):
                        gg = t * GPT + g
                        sel = sel8_sb[:, gg * SLOTS : (gg + 1) * SLOTS]
                        jobs.append((sel, a_sb[t][:, g * 2 * D : (g + 1) * 2 * D]))
                        jobs.append(
                            (
                                sel,
                                a_sb[t][
                                    :, 2 * D * GPT + g * 2 * D : 2 * D * GPT + (g + 1) * 2 * D
                                ],
                            )
                        )
                        jobs.append(
                            (sel, a_sb[t][:, RAW0 + g * 4 * D : RAW0 + g * 4 * D + 2 * D])
                        )
                        jobs.append(
                            (
                                sel,
                                a_sb[t][
                                    :, RAW0 + g * 4 * D + 2 * D : RAW0 + (g + 1) * 4 * D
                                ],
                            )
                        )
                # deferred merged matmuls from the previous tile
                jobs.extend(pend_merge)
                pend_merge = []
                if merged:
                    for g in range(GPT):
                        gg = t * GPT + g
                        pend_merge.append(
                            (
                                sel16_sb[:, gg * SLOTS : (gg + 1) * SLOTS],
                                m_sb[t][:, g * 2 * D : (g + 1) * 2 * D],
                            )
                        )
            jobs.extend(pend_merge)
            # remainder groups last: tiny, data long since arrived
            for k, r in enumerate(rems):
                gg = NGF + k
                for j in range(G // 2):
                    jobs.append(
                        (
                            sel8_sb[0:r, gg * SLOTS : (gg + 1) * SLOTS],
                            ar_sb[k][:, j * 2 * D : (j + 1) * 2 * D],
                        )
                    )

            for i, (lhsT, rhs) in enumerate(jobs):
                nc.tensor.matmul(
                    x_ps[:],
                    lhsT=lhsT,
                    rhs=rhs,
                    start=(i == 0),
                    stop=(i == len(jobs) - 1),
                )

            # ---- tail: out = q + x @ (hops*W) ----
            xh_sb = tpool.tile([SLOTS, D], F32)
            nc.vector.tensor_copy(out=xh_sb[:], in_=x_ps[:, 0:D])
            x2_sb = tpool.tile([SLOTS, D], F32)
            nc.vector.tensor_add(out=x2_sb[:], in0=xh_sb[:], in1=x_ps[:, D : 2 * D])
            xT_sb = tpool.tile([P, 2 * SLOTS], F16)
            for h in range(2):
                tp_ps = ps2pool.tile([P, SLOTS], F32)
                nc.tensor.transpose(
                    tp_ps[:], x2_sb[:, h * P : (h + 1) * P], id_sb
                )
                nc.vector.tensor_copy(
                    out=xT_sb[:, h * SLOTS : (h + 1) * SLOTS], in_=tp_ps[:]
                )
            out_ps = ps2pool.tile([SLOTS, D], F32)
            for h in range(2):
                nc.tensor.matmul(
                    out_ps[:],
                    lhsT=xT_sb[:, h * SLOTS : (h + 1) * SLOTS],
                    rhs=w_sb[:, h * D : (h + 1) * D],
                    start=(h == 0),
                    stop=(h == 1),
                )
            out_sb = tpool.tile([SLOTS, D], F32)
            nc.vector.tensor_add(out=out_sb[:], in0=q_sb, in1=out_ps[:])
            nc.sync.dma_start(out=out_d[:], in_=out_sb[:])

    nc.compile()
    return nc


def _prepare(sentences, masking, W, hops):
    """Host sharding: quantize valid rows to fp8, lane-pack, split into 8
    balanced contiguous spans, build per-core tile/selector arrays."""
    sentences = np.asarray(sentences)
    masking = np.asarray(masking)
    W = np.ascontiguousarray(np.asarray(W), dtype=np.float32)
    hops = int(np.asarray(hops))

    B, S, Dd = sentences.shape
    assert Dd == D
    lengths = masking.astype(np.int64).sum(axis=-1)  # [B]
    qidx = np.clip(lengths - 1, 0, S - 1)
    query = np.ascontiguousarray(
        sentences[np.arange(B), qidx], dtype=np.float32
    )  # [B, D]
    mem = np.clip(lengths - 1, 0, S).astype(np.int64)  # valid memory rows

    lanes_b = -(-mem // G)  # ceil
    lane_ofs = np.concatenate([[0], np.cumsum(lanes_b)])
    L = int(lane_ofs[-1])
    base = -(-L // N_CORES)
    fg = base // P  # full groups per core
    rem = base - fg * P
    rems = []
    while rem > 0:
        rems.append(min(rem, P))
        rem -= min(rem, P)
    rems = tuple(rems)
    # tiles: pairs of groups, but the last two tiles single-group so the
    # final PE burst after the last DMA byte is short
    head = max(fg - 2, 0)
    plan = [2] * (head // 2) + ([1] if head % 2 else []) + [1] * min(fg, 2)
    plan = tuple(plan)
    NGF = fg
    NG = NGF + len(rems)

    # quantize + lane-pack all batches into one [base*8, G, D] fp8 array
    Apad = np.zeros((base * N_CORES, G, D), dtype=NP8)
    rows_flat = Apad.reshape(-1, D)
    for b in range(B):
        m = int(mem[b])
        if m:
            rows_flat[lane_ofs[b] * G : lane_ofs[b] * G + m] = sentences[
                b, :m
            ].astype(NP8)

    # per-lane batch id
    lane_batch = np.repeat(np.arange(B), lanes_b)
    lane_batch = np.concatenate(
        [lane_batch, np.full(base * N_CORES - L, -1, dtype=np.int64)]
    )
    w16 = (W * np.float32(hops)).astype(np.float16).reshape(2, P, D)
    id_dev = np.eye(SLOTS, dtype=np.float32)

    in_maps = []
    core_slots: list[list[int]] = []
    for c in range(N_CORES):
        span = lane_batch[c * base : (c + 1) * base]
        bids: list[int] = []
        slot_of = {}
        for b in span:
            if b >= 0 and b not in slot_of:
                slot_of[b] = len(bids)
                bids.append(int(b))
        assert len(bids) <= SLOTS, f"core {c} needs {len(bids)} slots"
        core_slots.append(bids)

        sel8 = np.zeros((P, NG * SLOTS), dtype=NP8)
        sel16 = np.zeros((P, NGF * SLOTS), dtype=np.float16)
        for g in range(NG):
            g0 = g * P
            gl = P if g < NGF else rems[g - NGF]
            for p in range(gl):
                li = g0 + p
                if li < base and span[li] >= 0:
                    j = slot_of[int(span[li])]
                    sel8[p, g * SLOTS + j] = 1.0
                    if g < NGF:
                        sel16[p, g * SLOTS + j] = 1.0

        lanes = Apad[c * base : (c + 1) * base]  # [base, G, D]
        blocks = [sel8]
        g0 = 0
        for gpt in plan:
            lt = lanes[g0 * P : (g0 + gpt) * P].reshape(gpt, P, G, D)
            in0 = lt[:, :, 0:4:2, :].transpose(1, 0, 2, 3).reshape(P, -1)
            in1 = lt[:, :, 1:4:2, :].transpose(1, 0, 2, 3).reshape(P, -1)
            raw = lt[:, :, 4:8, :].transpose(1, 0, 2, 3).reshape(P, -1)
            blocks += [in0, in1, raw]
            g0 += gpt
        a_dev = np.ascontiguousarray(np.concatenate(blocks, axis=1))

        qmat = np.zeros((SLOTS, D), dtype=np.float32)
        for j, b in enumerate(bids):
            if int(lane_ofs[b]) // base == c:  # q rides with the owner core
                qmat[j] = query[b]
        c32 = np.concatenate([qmat, id_dev], axis=1)

        im = {
            "a": a_dev,
            "sel16": sel16,
            "w16": np.ascontiguousarray(w16.transpose(1, 0, 2).reshape(P, 2 * D)),
            "c32": np.ascontiguousarray(c32),
        }
        pos = fg * P
        for k, r in enumerate(rems):
            im[f"ar{k}"] = np.ascontiguousarray(
                lanes[pos : pos + r].reshape(r, G * D)
            )
            pos += r
        in_maps.append(im)
    return in_maps, core_slots, (plan, rems), query, mem, hops, B


def _run(sentences, masking, W, hops, trace=False):
    in_maps, core_slots, key, query, mem, hops_i, B = _prepare(
        sentences, masking, W, hops
    )
    if key not in _nc_cache:
        _nc_cache[key] = _build_bass(*key)
    nc = _nc_cache[key]
    res = run_bass_kernel_spmd(
        nc, in_maps, core_ids=list(range(N_CORES)), trace=trace
    )
    out = np.zeros((B, 1, D), dtype=np.float32)
    for c in range(N_CORES):
        r = res.results[c]["out"]
        for j, b in enumerate(core_slots[c]):
            out[b, 0] += r[j]
    for b in range(B):
        if mem[b] == 0:  # no memory rows: out = query (never had a slot)
            out[b, 0] = query[b]
    return out, res


def kernel(sentences, masking, W, hops):
    out, _ = _run(sentences, masking, W, hops)
    return out


# revision 18
# speedup vs baseline: 2.3018x; 1.0071x over previous
"""MemN2N kernel for 8 Trainium2 NeuronCores — fp8(e3m4) streaming version.

Math: the attention weights in the reference don't depend on the query, so the
module collapses to

    lengths[b] = sum(masking[b]);  q0[b] = sentences[b, lengths[b]-1]
    x[b]       = sum_{s < lengths[b]-1} sentences[b, s, :]
    out        = q0 + hops * (x @ W)                      # [B, 1, D]

The only heavy part is the masked row-sum x — pure HBM streaming.  Design:

* Rows are quantized host-side to float8_e3m4 (4 mantissa bits); the final
  rel-err this induces is ~1.3e-2 (measured against fp32), under the 2e-2
  tolerance, and it cuts DMA traffic 4x vs fp32.
* Lane packing: each batch's valid rows are split into lanes of G=8
  consecutive rows (last lane zero-padded).  The global lane stream is cut
  into 8 equal per-core spans at lane granularity, so core loads are balanced
  to within one lane and batches may split across cores (their partial sums
  are combined on the host; out = q + x@W is linear in x).
* A group = 128 lanes = one SBUF partition block.  Within a group, lane p's
  8 rows sit at partition p, depth slots 0..7 (chunks).  A per-group one-hot
  selector sel[p, j] (lane -> local batch slot) turns the PE into a segmented
  row-summer:  x_ps[16, 512] += sel[128,16].T @ chunk_pair[128, 512].
* PE streams 128 B/cycle of fp8, slightly slower than DMA (360 GB/s), so the
  DVE pre-adds depth pairs (0+1, 2+3) of every lane into fp16 "merged" chunks
  — half those rows then cost the PE half the cycles.  The LAST tile (and the
  remainder) stay all-raw so nothing on the end-of-stream critical path waits
  for the DVE; merged matmuls for tile t are issued after tile t+1's raw ones
  to give the DVE a full tile of slack.
* 512 KB DMA tiles (2 groups), alternating sync/scalar HWDGE queues;
  constants + remainder ride the gpsimd (SWDGE) queue; a burst of dummy
  matmuls on an uninitialized tile ramps the PE p-state during NEFF startup.
* Tail per core: fold hops into W (fp16), transpose x via the PE, two
  matmuls, add the gathered fp32 query rows, DMA [16, 256] out.
"""

import numpy as np
import ml_dtypes

import concourse.bass as bass
import concourse.mybir as mybir
from concourse import bacc
from concourse.bass_utils import run_bass_kernel_spmd
from concourse.tile import TileContext

N_CORES = 8
SLOTS = 16  # max distinct batches per core span
P = 128  # SBUF partitions = lanes per group
D = 256  # model dim
G = 8  # rows per lane (depth)
GPT = 2  # groups per full DMA tile (512 KB fp8)
LANES_PER_TILE = GPT * P
WARM = 10  # PE p-state warmup matmuls

F8 = mybir.dt.float8e3
F16 = mybir.dt.float16
F32 = mybir.dt.float32
NP8 = ml_dtypes.float8_e3m4

_nc_cache: dict = {}


def _build_bass(plan: tuple, rems: tuple):
    """One-core program: payload tiles of plan[t] groups each (the last two
    tiles are single-group so the end-of-stream PE burst is short) plus
    len(rems) partial remainder groups.  All tiles except the last two run
    the DVE pre-add on depths 0..3; the final tiles are all-raw."""
    T = len(plan)
    NGF = sum(plan)  # full groups
    NG = NGF + len(rems)
    n_merge_tiles = max(T - 2, 0)

    nc = bacc.Bacc(None)
    SEL_W = NG * SLOTS  # fp8 selector rides inside tile 0's dma
    coff = [0]
    for t_, g_ in enumerate(plan):
        coff.append(coff[-1] + (SEL_W if t_ == 0 else 0) + 8 * D * g_)
    a_d = nc.dram_tensor("a", [P, coff[-1]], F8, kind="ExternalInput")
    ar_d = [
        nc.dram_tensor(f"ar{k}", [r, G * D], F8, kind="ExternalInput")
        for k, r in enumerate(rems)
    ]
    sel16_d = nc.dram_tensor("sel16", [P, NGF * SLOTS], F16, kind="ExternalInput")
    w16_d = nc.dram_tensor("w16", [P, 2 * D], F16, kind="ExternalInput")
    c32_d = nc.dram_tensor("c32", [SLOTS, D + SLOTS], F32, kind="ExternalInput")
    out_d = nc.dram_tensor("out", [SLOTS, D], F32, kind="ExternalOutput")

    with TileContext(nc) as tc:
        with (
            tc.tile_pool(name="const", bufs=1) as cpool,
            tc.tile_pool(name="a", bufs=1) as apool,
            tc.tile_pool(name="m", bufs=1) as mpool,
            tc.tile_pool(name="acc", bufs=1, space=bass.MemorySpace.PSUM) as accpool,
            tc.tile_pool(name="wps", bufs=1, space=bass.MemorySpace.PSUM) as wpspool,
            tc.tile_pool(name="ps2", bufs=2, space=bass.MemorySpace.PSUM) as ps2pool,
            tc.tile_pool(name="tail", bufs=1) as tpool,
        ):
            # selectors + remainder go FIRST on the sync queue (tiny
            # transfers, ahead of the payload flood on the shared DMA
            # engines); payload tiles start immediately on scalar.  Tail-only
            # constants + the warm memset ride the gpsimd/SWDGE queue.
            sel16_sb = cpool.tile([P, NGF * SLOTS], F16)
            nc.sync.dma_start(out=sel16_sb[:], in_=sel16_d[:])
            ar_sb = []
            for k, r in enumerate(rems):
                t_ = cpool.tile([r, G * D], F8, tag=f"ar{k}")
                nc.sync.dma_start(out=t_[:], in_=ar_d[k][:])
                ar_sb.append(t_)
            warm_sb = cpool.tile([P, 512], F16)
            nc.gpsimd.memset(warm_sb[:], 0.0)
            w_sb = cpool.tile([P, 2 * D], F16)
            nc.gpsimd.dma_start(out=w_sb[:], in_=w16_d[:])
            c32_sb = cpool.tile([SLOTS, D + SLOTS], F32)
            nc.gpsimd.dma_start(out=c32_sb[:], in_=c32_d[:])
            q_sb = c32_sb[:, 0:D]
            id_sb = c32_sb[:, D : D + SLOTS]

            # PE p-state warmup: throwaway psum bank, never read
            warm_ps = wpspool.tile([SLOTS, 512], F32)
            for _ in range(WARM):
                nc.tensor.matmul(
                    warm_ps[:],
                    lhsT=warm_sb[:, 0:SLOTS],
                    rhs=warm_sb[:],
                    start=True,
                    stop=True,
                )

            # ---- streamed masked row-sum (single psum accumulation chain).
            # Jobs are collected in PE issue order so start/stop land on the
            # first/last matmul of the chain.
            x_ps = accpool.tile([SLOTS, 2 * D], F32)
            jobs = []  # (lhsT, rhs) in PE issue order

            a_sb = [None] * T
            m_sb = [None] * n_merge_tiles
            pend_merge = []  # deferred merged-matmul jobs, one tile behind
            for t in range(T):
                a_sb[t] = apool.tile([P, TILE_C], F8)
                nc.scalar.dma_start(out=a_sb[t][:], in_=a_d[t])
                merged = t < n_merge_tiles
                if merged:
                    m_sb[t] = mpool.tile([P, 2 * D * GPT], F16)
                    nc.vector.tensor_add(
                        out=m_sb[t][:],
                        in0=a_sb[t][:, 0 : 2 * D * GPT],
                        in1=a_sb[t][:, 2 * D * GPT : RAW0],
                    )
                # raw matmuls of this tile (depths 4..7 from the raw block)
                for g in range(GPT):
                    gg = t * GPT + g
                    sel = sel8_sb[:, gg * SLOTS : (gg + 1) * SLOTS]
                    lo = RAW0 + g * 4 * D
                    if merged:
                        for j in range(2):
                            jobs.append(
                                (sel, a_sb[t][:, lo + j * 2 * D : lo + (j + 1) * 2 * D])
                            )
                # all-raw tile: depths 0..3 also stream straight from the
                # in0/in1 blocks (host emits the same [in0|in1|raw] layout)
                if not merged:
                    for g in range(GPT):
                        gg = t * GPT + g
                        sel = sel8_sb[:, gg * SLOTS : (gg + 1) * SLOTS]
                        jobs.append((sel, a_sb[t][:, g * 2 * D : (g + 1) * 2 * D]))
                        jobs.append(
                            (
                                sel,
                                a_sb[t][
                                    :, 2 * D * GPT + g * 2 * D : 2 * D * GPT + (g + 1) * 2 * D
                                ],
                            )
                        )
                        jobs.append(
                            (sel, a_sb[t][:, RAW0 + g * 4 * D : RAW0 + g * 4 * D + 2 * D])
                        )
                        jobs.append(
                            (
                                sel,
                                a_sb[t][
                                    :, RAW0 + g * 4 * D + 2 * D : RAW0 + (g + 1) * 4 * D
                                ],
                            )
                        )
                # deferred merged matmuls from the previous tile
                jobs.extend(pend_merge)
                pend_merge = []
                if merged:
                    for g in range(GPT):
                        gg = t * GPT + g
                        pend_merge.append(
                            (
                                sel16_sb[:, gg * SLOTS : (gg + 1) * SLOTS],
                                m_sb[t][:, g * 2 * D : (g + 1) * 2 * D],
                            )
                        )
            jobs.extend(pend_merge)
            # remainder groups last: tiny, data long since arrived
            for k, r in enumerate(rems):
                gg = NGF + k
                for j in range(G // 2):
                    jobs.append(
                        (
                            sel8_sb[0:r, gg * SLOTS : (gg + 1) * SLOTS],
                            ar_sb[k][:, j * 2 * D : (j + 1) * 2 * D],
                        )
                    )

            for i, (lhsT, rhs) in enumerate(jobs):
                nc.tensor.matmul(
                    x_ps[:],
                    lhsT=lhsT,
                    rhs=rhs,
                    start=(i == 0),
                    stop=(i == len(jobs) - 1),
                )

            # ---- tail: out = q + x @ (hops*W) ----
            xh_sb = tpool.tile([SLOTS, D], F32)
            nc.vector.tensor_copy(out=xh_sb[:], in_=x_ps[:, 0:D])
            x2_sb = tpool.tile([SLOTS, D], F32)
            nc.vector.tensor_add(out=x2_sb[:], in0=xh_sb[:], in1=x_ps[:, D : 2 * D])
            xT_sb = tpool.tile([P, 2 * SLOTS], F16)
            for h in range(2):
                tp_ps = ps2pool.tile([P, SLOTS], F32)
                nc.tensor.transpose(
                    tp_ps[:], x2_sb[:, h * P : (h + 1) * P], id_sb
                )
                nc.vector.tensor_copy(
                    out=xT_sb[:, h * SLOTS : (h + 1) * SLOTS], in_=tp_ps[:]
                )
            out_ps = ps2pool.tile([SLOTS, D], F32)
            for h in range(2):
                nc.tensor.matmul(
                    out_ps[:],
                    lhsT=xT_sb[:, h * SLOTS : (h + 1) * SLOTS],
                    rhs=w_sb[:, h * D : (h + 1) * D],
                    start=(h == 0),
                    stop=(h == 1),
                )
            out_sb = tpool.tile([SLOTS, D], F32)
            nc.vector.tensor_add(out=out_sb[:], in0=q_sb, in1=out_ps[:])
            nc.sync.dma_start(out=out_d[:], in_=out_sb[:])

    nc.compile()
    return nc


def _prepare(sentences, masking, W, hops):
    """Host sharding: quantize valid rows to fp8, lane-pack, split into 8
    balanced contiguous spans, build per-core tile/selector arrays."""
    sentences = np.asarray(sentences)
    masking = np.asarray(masking)
    W = np.ascontiguousarray(np.asarray(W), dtype=np.float32)
    hops = int(np.asarray(hops))

    B, S, Dd = sentences.shape
    assert Dd == D
    lengths = masking.astype(np.int64).sum(axis=-1)  # [B]
    qidx = np.clip(lengths - 1, 0, S - 1)
    query = np.ascontiguousarray(
        sentences[np.arange(B), qidx], dtype=np.float32
    )  # [B, D]
    mem = np.clip(lengths - 1, 0, S).astype(np.int64)  # valid memory rows

    lanes_b = -(-mem // G)  # ceil
    lane_ofs = np.concatenate([[0], np.cumsum(lanes_b)])
    L = int(lane_ofs[-1])
    base = -(-L // N_CORES)
    fg = base // P  # full groups per core
    rem = base - fg * P
    rems = []
    while rem > 0:
        rems.append(min(rem, P))
        rem -= min(rem, P)
    rems = tuple(rems)
    # tiles: pairs of groups, but the last two tiles single-group so the
    # final PE burst after the last DMA byte is short
    head = max(fg - 2, 0)
    plan = [2] * (head // 2) + ([1] if head % 2 else []) + [1] * min(fg, 2)
    plan = tuple(plan)
    NGF = fg
    NG = NGF + len(rems)

    # quantize + lane-pack all batches into one [base*8, G, D] fp8 array
    Apad = np.zeros((base * N_CORES, G, D), dtype=NP8)
    rows_flat = Apad.reshape(-1, D)
    for b in range(B):
        m = int(mem[b])
        if m:
            rows_flat[lane_ofs[b] * G : lane_ofs[b] * G + m] = sentences[
                b, :m
            ].astype(NP8)

    # per-lane batch id
    lane_batch = np.repeat(np.arange(B), lanes_b)
    lane_batch = np.concatenate(
        [lane_batch, np.full(base * N_CORES - L, -1, dtype=np.int64)]
    )
    w16 = (W * np.float32(hops)).astype(np.float16).reshape(2, P, D)
    id_dev = np.eye(SLOTS, dtype=np.float32)

    in_maps = []
    core_slots: list[list[int]] = []
    for c in range(N_CORES):
        span = lane_batch[c * base : (c + 1) * base]
        bids: list[int] = []
        slot_of = {}
        for b in span:
            if b >= 0 and b not in slot_of:
                slot_of[b] = len(bids)
                bids.append(int(b))
        assert len(bids) <= SLOTS, f"core {c} needs {len(bids)} slots"
        core_slots.append(bids)

        sel8 = np.zeros((P, NG * SLOTS), dtype=NP8)
        sel16 = np.zeros((P, NGF * SLOTS), dtype=np.float16)
        for g in range(NG):
            g0 = g * P
            gl = P if g < NGF else rems[g - NGF]
            for p in range(gl):
                li = g0 + p
                if li < base and span[li] >= 0:
                    j = slot_of[int(span[li])]
                    sel8[p, g * SLOTS + j] = 1.0
                    if g < NGF:
                        sel16[p, g * SLOTS + j] = 1.0

        lanes = Apad[c * base : (c + 1) * base]  # [base, G, D]
        blocks = [sel8]
        g0 = 0
        for gpt in plan:
            lt = lanes[g0 * P : (g0 + gpt) * P].reshape(gpt, P, G, D)
            in0 = lt[:, :, 0:4:2, :].transpose(1, 0, 2, 3).reshape(P, -1)
            in1 = lt[:, :, 1:4:2, :].transpose(1, 0, 2, 3).reshape(P, -1)
            raw = lt[:, :, 4:8, :].transpose(1, 0, 2, 3).reshape(P, -1)
            blocks += [in0, in1, raw]
            g0 += gpt
        a_dev = np.ascontiguousarray(np.concatenate(blocks, axis=1))

        qmat = np.zeros((SLOTS, D), dtype=np.float32)
        for j, b in enumerate(bids):
            if int(lane_ofs[b]) // base == c:  # q rides with the owner core
                qmat[j] = query[b]
        c32 = np.concatenate([qmat, id_dev], axis=1)

        im = {
            "a": a_dev,
            "sel16": sel16,
            "w16": np.ascontiguousarray(w16.transpose(1, 0, 2).reshape(P, 2 * D)),
            "c32": np.ascontiguousarray(c32),
        }
        pos = fg * P
        for k, r in enumerate(rems):
            im[f"ar{k}"] = np.ascontiguousarray(
                lanes[pos : pos + r].reshape(r, G * D)
            )
            pos += r
        in_maps.append(im)
    return in_maps, core_slots, (plan, rems), query, mem, hops, B


def _run(sentences, masking, W, hops, trace=False):
    in_maps, core_slots, key, query, mem, hops_i, B = _prepare(
        sentences, masking, W, hops
    )
    if key not in _nc_cache:
        _nc_cache[key] = _build_bass(*key)
    nc = _nc_cache[key]
    res = run_bass_kernel_spmd(
        nc, in_maps, core_ids=list(range(N_CORES)), trace=trace
    )
    out = np.zeros((B, 1, D), dtype=np.float32)
    for c in range(N_CORES):
        r = res.results[c]["out"]
        for j, b in enumerate(core_slots[c]):
            out[b, 0] += r[j]
    for b in range(B):
        if mem[b] == 0:  # no memory rows: out = query (never had a slot)
            out[b, 0] = query[b]
    return out, res


def kernel(sentences, masking, W, hops):
    out, _ = _run(sentences, masking, W, hops)
    return out
